# revision 1
# baseline (speedup 1.0000x reference)
"""Trainium2 Bass kernel for nn_Bidir_Attention (top-k masked bidirectional
cross-attention).

Data-parallel over batch: each of the 8 NeuronCores processes one batch
element end-to-end (QKV GEMM, scores, softmax, exact top-16 mask via
max8+match_replace, masked AV). W_qkv is replicated.

Self-contained: hardcodes B=8, N=2048, D=1024, topk=16.
"""

import sys

import numpy as np

for _p in ("/opt/trn_rl_repo", "/root/.axon_site/_ro/trn_rl_repo"):
    if _p not in sys.path:
        sys.path.append(_p)

import concourse.bacc as bacc
import concourse.mybir as mybir
from concourse.tile import TileContext
from concourse.masks import make_identity
from concourse.bass_utils import run_bass_kernel_spmd

B = 8
N = 2048
D = 1024
NT = N // 128          # 16 row tiles
DT = D // 128          # 8 contraction tiles
TOPK = 16
SCALE = float(1.0 / np.sqrt(D))
NEG = -1e30
F32 = mybir.dt.float32
F32R = mybir.dt.float32r


def _phase_a(nc, pools, x_dram, ident_r, wqk, wv, qt_dram, kt_dram, v_dram):
    """QKV GEMM for one feature: writes Q^T (pre-scaled), K^T (both [D,N])
    and V ([N,D]) to DRAM scratch. x is consumed transposed via PE."""
    sb, ps = pools
    for j in range(4):                      # supertiles of 512 rows
        xs = []
        for nsub in range(4):
            x = sb.tile([128, D], F32, tag=f"x{nsub}", bufs=2)
            nc.sync.dma_start(
                out=x[:], in_=x_dram.ap()[j * 512 + nsub * 128: j * 512 + (nsub + 1) * 128, :])
            xs.append(x)
        xT = sb.tile([128, DT, 512], F32, tag="xT", bufs=1)
        xh = sb.tile([128, DT, 512], mybir.dt.bfloat16, tag="xh", bufs=1)
        xl = sb.tile([128, DT, 512], mybir.dt.bfloat16, tag="xl", bufs=1)
        for nsub in range(4):
            for di in range(DT):
                tp = ps.tile([128, 128], F32, tag="tp")
                nc.tensor.transpose(tp[:], xs[nsub][:, di * 128:(di + 1) * 128], ident_r[:])
                sl = (di, slice(nsub * 128, (nsub + 1) * 128))
                if (nsub * DT + di) % 2:
                    nc.vector.tensor_copy(xT[:, sl[0], sl[1]], tp[:])
                else:
                    nc.scalar.copy(xT[:, sl[0], sl[1]], tp[:])
                nc.vector.tensor_copy(xh[:, sl[0], sl[1]], tp[:])
                nc.vector.tensor_sub(xl[:, sl[0], sl[1]], tp[:], xh[:, sl[0], sl[1]])
        # Q^T and K^T: [dout 128-tile, n 512] pieces
        for t in range(16):
            qk_ps = ps.tile([128, 512], F32, tag="qk_ps")
            for di in range(DT):
                nc.tensor.matmul(qk_ps[:], wqk[di][t][:], xT[:, di, :],
                                 start=(di == 0), stop=(di == DT - 1))
            o = sb.tile([128, 512], F32, tag="qko")
            if t < 8:
                nc.scalar.mul(o[:], qk_ps[:], SCALE)   # fold in 1/sqrt(D)
                dst = qt_dram
                r0 = t * 128
            else:
                nc.vector.tensor_copy(o[:], qk_ps[:])
                dst = kt_dram
                r0 = (t - 8) * 128
            nc.gpsimd.dma_start(
                out=dst.ap()[r0:r0 + 128, j * 512:(j + 1) * 512], in_=o[:])
        # V: natural layout [n 128-tile, dout 512] pieces
        for nsub in range(4):
            for c in range(2):
                v_ps = ps.tile([128, 512], F32, tag="v_ps")
                nsl = slice(nsub * 128, (nsub + 1) * 128)
                n_mm = DT * 3
                i_mm = 0
                for di in range(DT):
                    for lhs_t, rhs_t in ((xh, 0), (xh, 1), (xl, 0)):
                        nc.tensor.matmul(v_ps[:], lhs_t[:, di, nsl],
                                         wv[di][c][rhs_t][:],
                                         start=(i_mm == 0), stop=(i_mm == n_mm - 1))
                        i_mm += 1
                o = sb.tile([128, 512], F32, tag="vo")
                if (nsub * 2 + c) % 2:
                    nc.vector.tensor_copy(o[:], v_ps[:])
                else:
                    nc.scalar.copy(o[:], v_ps[:])
                nc.gpsimd.dma_start(
                    out=v_dram.ap()[j * 512 + nsub * 128: j * 512 + (nsub + 1) * 128,
                                    c * 512:(c + 1) * 512],
                    in_=o[:])


def _phase_b(nc, pools, ident_f, qt_dram, kt_dram, v_dram, out_dram):
    """One attention direction: S = Q^T.T @ K^T (pre-scaled), softmax row
    stats, exact top-16 mask, masked AV, 1/Z renormalization."""
    sbr, sb, ps = pools
    # residents
    kt = []
    for di in range(DT):
        t = sbr.tile([128, N], F32, tag=f"kt{di}", name=f"kt{di}")
        nc.sync.dma_start(out=t[:], in_=kt_dram.ap()[di * 128:(di + 1) * 128, :])
        kt.append(t)
    vres = []
    for nt in range(NT):
        t = sb.tile([128, D], F32, tag="vtmp", name="vtmp")
        nc.sync.dma_start(out=t[:], in_=v_dram.ap()[nt * 128:(nt + 1) * 128, :])
        vh = sbr.tile([128, D], mybir.dt.bfloat16, tag=f"vh{nt}", name=f"vh{nt}")
        vl = sbr.tile([128, D], mybir.dt.bfloat16, tag=f"vl{nt}", name=f"vl{nt}")
        nc.vector.tensor_copy(vh[:], t[:])
        nc.vector.tensor_sub(vl[:], t[:], vh[:])
        vres.append((vh, vl))

    for qi in range(NT):
        qts = []
        for di in range(DT):
            t = sb.tile([128, 128], F32, tag=f"qt{di}", name=f"qt{di}")
            nc.sync.dma_start(
                out=t[:], in_=qt_dram.ap()[di * 128:(di + 1) * 128, qi * 128:(qi + 1) * 128])
            qts.append(t)
        ssb = sb.tile([128, N], F32, tag="ssb")
        for half in range(2):
            s_ps = ps.tile([128, N // 2], F32, tag="s_ps", bufs=2)
            for di in range(DT):
                for c in range(2):
                    nc.tensor.matmul(s_ps[:, c * 512:(c + 1) * 512], qts[di][:],
                                     kt[di][:, half * 1024 + c * 512:
                                            half * 1024 + (c + 1) * 512],
                                     start=(di == 0), stop=(di == DT - 1))
            nc.vector.tensor_copy(ssb[:, half * 1024:(half + 1) * 1024], s_ps[:])

        m0 = sb.tile([128, 8], F32, tag="m0")
        nc.vector.max(out=m0[:], in_=ssb[:])
        nm = sb.tile([128, 1], F32, tag="nm")
        nc.vector.tensor_scalar_mul(nm[:], m0[:, 0:1], -1.0)
        p = sb.tile([128, N], F32, tag="p")
        z = sb.tile([128, 1], F32, tag="z")
        nc.scalar.activation(p[:], ssb[:], mybir.ActivationFunctionType.Exp,
                             bias=nm[:], scale=1.0, accum_out=z[:])
        iz = sb.tile([128, 1], F32, tag="iz")
        nc.vector.reciprocal(iz[:], z[:])
        # exact top-16: two rounds of max8 + match_replace (in place on ssb,
        # which the Exp above has already consumed)
        nc.vector.match_replace(out=ssb[:], in_to_replace=m0[:], in_values=ssb[:],
                                imm_value=NEG)
        m8 = sb.tile([128, 8], F32, tag="m8")
        nc.vector.max(out=m8[:], in_=ssb[:])
        nc.vector.match_replace(out=ssb[:], in_to_replace=m8[:], in_values=ssb[:],
                                imm_value=NEG)
        # A = exp(S - m) where selected else 0   (in place on p)
        nc.vector.scalar_tensor_tensor(out=p[:], in0=ssb[:], scalar=NEG, in1=p[:],
                                       op0=mybir.AluOpType.is_equal,
                                       op1=mybir.AluOpType.mult)
        # transpose A tiles for the AV matmul
        ats = []
        for kt_i in range(NT):
            tp = ps.tile([128, 128], F32, tag="tp2")
            nc.tensor.transpose(tp[:], p[:, kt_i * 128:(kt_i + 1) * 128], ident_f[:])
            ah = sbr.tile([128, 128], mybir.dt.bfloat16, tag=f"ah{kt_i}", name=f"ah{kt_i}")
            al = sbr.tile([128, 128], mybir.dt.bfloat16, tag=f"al{kt_i}", name=f"al{kt_i}")
            if kt_i % 2:
                nc.vector.tensor_copy(ah[:], tp[:])
            else:
                nc.scalar.copy(ah[:], tp[:])
            nc.vector.tensor_sub(al[:], tp[:], ah[:])
            ats.append((ah, al))
        osb = sb.tile([128, D], F32, tag="osb")
        for h in range(2):
            o_ps = ps.tile([128, 512], F32, tag="o_ps")
            hs = slice(h * 512, (h + 1) * 512)
            n_mm = NT * 3
            i_mm = 0
            for kt_i in range(NT):
                ah, al = ats[kt_i]
                vh, vl = vres[kt_i]
                for lhs_t, rhs_t in ((ah, vh), (ah, vl), (al, vh)):
                    nc.tensor.matmul(o_ps[:], lhs_t[:], rhs_t[:, hs],
                                     start=(i_mm == 0), stop=(i_mm == n_mm - 1))
                    i_mm += 1
            nc.vector.tensor_scalar_mul(osb[:, hs], o_ps[:], iz[:])
        nc.gpsimd.dma_start(out=out_dram.ap()[qi * 128:(qi + 1) * 128, :], in_=osb[:])


def build():
    nc = bacc.Bacc()
    f1 = nc.declare_dram_parameter("feature1", [N, D], F32, isOutput=False)
    f2 = nc.declare_dram_parameter("feature2", [N, D], F32, isOutput=False)
    w = nc.declare_dram_parameter("w_qkv", [D, 3 * D], F32, isOutput=False)
    out1 = nc.declare_dram_parameter("out1", [N, D], F32, isOutput=True)
    out2 = nc.declare_dram_parameter("out2", [N, D], F32, isOutput=True)

    q1t = nc.dram_tensor("q1t", [D, N], F32)
    k1t = nc.dram_tensor("k1t", [D, N], F32)
    v1 = nc.dram_tensor("v1", [N, D], F32)
    q2t = nc.dram_tensor("q2t", [D, N], F32)
    k2t = nc.dram_tensor("k2t", [D, N], F32)
    v2 = nc.dram_tensor("v2", [N, D], F32)

    with TileContext(nc) as tc:
        with tc.tile_pool(name="const", bufs=1) as constp:
            ident_f = constp.tile([128, 128], F32, tag="id_f")
            make_identity(nc, ident_f[:])

            with (
                tc.tile_pool(name="wpool", bufs=1) as wp,
                tc.tile_pool(name="apool", bufs=1) as asb,
                tc.tile_pool(name="apsum", bufs=2, space="PSUM") as aps,
            ):
                wqk = []
                for di in range(DT):
                    row = []
                    for t in range(16):
                        wt = wp.tile([128, 128], F32, tag=f"w{di}_{t}")
                        nc.sync.dma_start(
                            out=wt[:],
                            in_=w.ap()[di * 128:(di + 1) * 128, t * 128:(t + 1) * 128])
                        row.append(wt)
                    wqk.append(row)
                wv = []
                for di in range(DT):
                    row = []
                    for c in range(2):
                        wt = wp.tile([128, 512], F32, tag=f"wvt{di}_{c}", name=f"wvt{di}_{c}")
                        nc.sync.dma_start(
                            out=wt[:],
                            in_=w.ap()[di * 128:(di + 1) * 128,
                                       2048 + c * 512:2048 + (c + 1) * 512])
                        wh = wp.tile([128, 512], mybir.dt.bfloat16, tag=f"wvh{di}_{c}", name=f"wvh{di}_{c}")
                        wl = wp.tile([128, 512], mybir.dt.bfloat16, tag=f"wvl{di}_{c}", name=f"wvl{di}_{c}")
                        nc.vector.tensor_copy(wh[:], wt[:])
                        nc.vector.tensor_sub(wl[:], wt[:], wh[:])
                        row.append((wh, wl))
                    wv.append(row)
                _phase_a(nc, (asb, aps), f1, ident_f, wqk, wv, q1t, k1t, v1)
                _phase_a(nc, (asb, aps), f2, ident_f, wqk, wv, q2t, k2t, v2)

            with (
                tc.tile_pool(name="bpool", bufs=1) as bsb,
                tc.tile_pool(name="bwork", bufs=2) as bwk,
                tc.tile_pool(name="bpsum", bufs=2, space="PSUM") as bps,
            ):
                _phase_b(nc, (bsb, bwk, bps), ident_f, q1t, k2t, v2, out1)
                _phase_b(nc, (bsb, bwk, bps), ident_f, q2t, k1t, v1, out2)
    return nc


_NC_CACHE = None


def _get_nc():
    global _NC_CACHE
    if _NC_CACHE is None:
        _NC_CACHE = build()
        _NC_CACHE.finalize()
    return _NC_CACHE


def kernel(feature1, feature2, W_qkv, topk):
    assert int(topk) == TOPK, f"kernel hardcodes topk=16, got {topk}"
    f1 = np.ascontiguousarray(np.asarray(feature1), dtype=np.float32)
    f2 = np.ascontiguousarray(np.asarray(feature2), dtype=np.float32)
    w = np.ascontiguousarray(np.asarray(W_qkv), dtype=np.float32)
    assert f1.shape == (B, N, D) and f2.shape == (B, N, D) and w.shape == (D, 3 * D)

    nc = _get_nc()
    in_maps = [{"feature1": f1[b], "feature2": f2[b], "w_qkv": w} for b in range(B)]
    try:
        res = run_bass_kernel_spmd(nc, in_maps, list(range(B))).results
    except Exception:
        # transient device faults have been observed; one retry on a fresh
        # execution usually clears them
        res = run_bass_kernel_spmd(nc, in_maps, list(range(B))).results
    o1 = np.stack([res[b]["out1"] for b in range(B)]).astype(np.float32)
    o2 = np.stack([res[b]["out2"] for b in range(B)]).astype(np.float32)
    return o1, o2


if __name__ == "__main__":
    f1 = np.load("/root/problem/cache/f1.npy")
    f2 = np.load("/root/problem/cache/f2.npy")
    w = np.load("/root/problem/cache/W.npy")
    o1, o2 = kernel(f1, f2, w, 16)
    r1 = np.load("/root/problem/cache/r1.npy")
    r2 = np.load("/root/problem/cache/r2.npy")
    for nm, o, r in (("2to1", o1, r1), ("1to2", o2, r2)):
        err = np.abs(o - r).max()
        rel = err / np.abs(r).max()
        print(f"{nm}: absmax_err={err:.3e} rel={rel:.3e}")



# revision 11
# speedup vs baseline: 7.3448x; 7.3448x over previous
"""Trainium2 Bass kernel for nn_Bidir_Attention (top-k masked bidirectional
cross-attention).

Data-parallel over batch: each of the 8 NeuronCores processes one batch
element end-to-end (QKV GEMM, scores, softmax, exact top-16 mask via
max8+match_replace, masked AV). W_qkv is replicated.

Precision policy (validated against the reference's jax-on-neuron run):
- The S path (x -> Q,K -> S) is full fp32 PE matmul with d-ascending PSUM
  accumulation, bit-matching the reference's compiled matmuls. This is
  mandatory: the top-16 selection flips on ~1e-6 S perturbations and a
  single flipped row can exceed the 2e-2 gate.
- The V path (V GEMM, A transpose, AV) is single bf16: it only scales the
  output smoothly (measured worst-case 4.6e-3 rel in simulation).

Self-contained: hardcodes B=8, N=2048, D=1024, topk=16.
"""

import sys

import numpy as np

for _p in ("/opt/trn_rl_repo", "/root/.axon_site/_ro/trn_rl_repo"):
    if _p not in sys.path:
        sys.path.append(_p)

import concourse.bacc as bacc
import concourse.mybir as mybir
from concourse.tile import TileContext
from concourse.masks import make_identity
from concourse.bass_utils import run_bass_kernel_spmd

B = 8
N = 2048
D = 1024
NT = N // 128          # 16 row tiles
DT = D // 128          # 8 contraction tiles
TOPK = 16
SCALE = float(1.0 / np.sqrt(D))
NEG = -1e30
F32 = mybir.dt.float32
BF16 = mybir.dt.bfloat16


def _load_x(nc, sb, x_dram, j):
    """Issue the 4 x-tile DMAs for supertile j."""
    xs = []
    for nsub in range(4):
        x = sb.tile([128, D], F32, tag=f"x{nsub}", bufs=2)
        nc.sync.dma_start(
            out=x[:], in_=x_dram.ap()[j * 512 + nsub * 128: j * 512 + (nsub + 1) * 128, :])
        xs.append(x)
    return xs


def _phase_a(nc, pools, x_dram, ident_r, wqk, wv, qt_dram, kt_dram, v_dram,
             xs0=None):
    """QKV GEMM for one feature: writes Q^T (pre-scaled, fp32), K^T (fp32,
    both [D,N]) and V (bf16, [N,D]) to DRAM scratch. x is consumed
    transposed via PE. The Q/K path is bit-exact fp32; V is single bf16."""
    sb, ps = pools
    for j in range(4):                      # supertiles of 512 rows
        xs = xs0 if (j == 0 and xs0 is not None) else _load_x(nc, sb, x_dram, j)
        xT = sb.tile([128, DT, 512], F32, tag="xT", bufs=2)
        xh = sb.tile([128, DT, 512], BF16, tag="xh", bufs=2)
        for nsub in range(4):
            for di in range(DT):
                tp = ps.tile([128, 128], F32, tag="tp")
                nc.tensor.transpose(tp[:], xs[nsub][:, di * 128:(di + 1) * 128], ident_r[:])
                sl = (di, slice(nsub * 128, (nsub + 1) * 128))
                if (nsub * DT + di) % 2:
                    nc.vector.tensor_copy(xT[:, sl[0], sl[1]], tp[:])
                else:
                    nc.scalar.copy(xT[:, sl[0], sl[1]], tp[:])
                nc.vector.tensor_copy(xh[:, sl[0], sl[1]], tp[:])
        # Q^T and K^T: [dout 128-tile, n 512] pieces — fp32, d-ascending
        for t in range(16):
            qk_ps = ps.tile([128, 512], F32, tag="qk_ps")
            for di in range(DT):
                nc.tensor.matmul(qk_ps[:], wqk[di][:, t * 128:(t + 1) * 128],
                                 xT[:, di, :],
                                 start=(di == 0), stop=(di == DT - 1))
            o = sb.tile([128, 512], F32, tag="qko")
            if t < 8:
                nc.scalar.mul(o[:], qk_ps[:], SCALE)   # fold in 1/sqrt(D)
                dst = qt_dram
                r0 = t * 128
            else:
                nc.vector.tensor_copy(o[:], qk_ps[:])
                dst = kt_dram
                r0 = (t - 8) * 128
            nc.gpsimd.dma_start(
                out=dst.ap()[r0:r0 + 128, j * 512:(j + 1) * 512], in_=o[:])
        # V: natural layout [n 128-tile, dout 512] pieces — single bf16
        for nsub in range(4):
            for c in range(2):
                v_ps = ps.tile([128, 512], F32, tag="v_ps")
                nsl = slice(nsub * 128, (nsub + 1) * 128)
                for di in range(DT):
                    nc.tensor.matmul(v_ps[:], xh[:, di, nsl],
                                     wv[di][:, c * 512:(c + 1) * 512],
                                     start=(di == 0), stop=(di == DT - 1))
                o = sb.tile([128, 512], BF16, tag="vo")
                if (nsub * 2 + c) % 2:
                    nc.vector.tensor_copy(o[:], v_ps[:])
                else:
                    nc.scalar.copy(o[:], v_ps[:])
                nc.gpsimd.dma_start(
                    out=v_dram.ap()[j * 512 + nsub * 128: j * 512 + (nsub + 1) * 128,
                                    c * 512:(c + 1) * 512],
                    in_=o[:])


def _phase_b(nc, pools, ident_f, qt_dram, kt_dram, v_dram, out_dram):
    """One attention direction: S = Q^T.T @ K^T (pre-scaled, fp32), softmax
    row stats, exact top-16 mask, masked AV in bf16, 1/Z renormalization."""
    sbr, sb, ps = pools
    # residents
    kt = []
    for di in range(DT):
        t = sbr.tile([128, N], F32, tag=f"kt{di}", name=f"kt{di}")
        nc.sync.dma_start(out=t[:], in_=kt_dram.ap()[di * 128:(di + 1) * 128, :])
        kt.append(t)
    vres = []
    for nt in range(NT):
        t = sbr.tile([128, D], BF16, tag=f"vh{nt}", name=f"vh{nt}")
        nc.sync.dma_start(out=t[:], in_=v_dram.ap()[nt * 128:(nt + 1) * 128, :])
        vres.append(t)

    for qi in range(NT):
        # all 8 Q^T blocks for this q-tile in one 512KB DMA:
        # qt_dram[(di p), qcols] -> [p, di, qcols]
        qts = sb.tile([128, DT, 128], F32, tag="qts", bufs=2)
        nc.sync.dma_start(
            out=qts[:],
            in_=qt_dram.ap()[:, qi * 128:(qi + 1) * 128]
                .rearrange("(di p) c -> p di c", p=128))
        ssb = sb.tile([128, N], F32, tag="ssb")
        for half in range(2):
            s_ps = ps.tile([128, N // 2], F32, tag="s_ps", bufs=2)
            for di in range(DT):
                for c in range(2):
                    nc.tensor.matmul(s_ps[:, c * 512:(c + 1) * 512], qts[:, di, :],
                                     kt[di][:, half * 1024 + c * 512:
                                            half * 1024 + (c + 1) * 512],
                                     start=(di == 0), stop=(di == DT - 1))
            nc.vector.tensor_copy(ssb[:, half * 1024:(half + 1) * 1024], s_ps[:])

        m0 = sb.tile([128, 8], F32, tag="m0")
        nc.vector.max(out=m0[:], in_=ssb[:])
        nm = sb.tile([128, 1], F32, tag="nm")
        nc.vector.tensor_scalar_mul(nm[:], m0[:, 0:1], -1.0)
        p = sb.tile([128, N], F32, tag="p")
        z = sb.tile([128, 1], F32, tag="z")
        nc.scalar.activation(p[:], ssb[:], mybir.ActivationFunctionType.Exp,
                             bias=nm[:], scale=1.0, accum_out=z[:])
        iz = sb.tile([128, 1], F32, tag="iz")
        nc.vector.reciprocal(iz[:], z[:])
        # exact top-16: two rounds of max8 + match_replace (in place on ssb,
        # which the Exp above has already consumed)
        nc.vector.match_replace(out=ssb[:], in_to_replace=m0[:], in_values=ssb[:],
                                imm_value=NEG)
        m8 = sb.tile([128, 8], F32, tag="m8")
        nc.vector.max(out=m8[:], in_=ssb[:])
        nc.vector.match_replace(out=ssb[:], in_to_replace=m8[:], in_values=ssb[:],
                                imm_value=NEG)
        # A = exp(S - m) where selected else 0   (in place on p)
        nc.vector.scalar_tensor_tensor(out=p[:], in0=ssb[:], scalar=NEG, in1=p[:],
                                       op0=mybir.AluOpType.is_equal,
                                       op1=mybir.AluOpType.mult)
        # transpose A tiles for the AV matmul (fp32 PE transpose, cast to
        # bf16 during the PSUM->SBUF copy)
        ats = []
        for kt_i in range(NT):
            tp = ps.tile([128, 128], F32, tag="tp2")
            nc.tensor.transpose(tp[:], p[:, kt_i * 128:(kt_i + 1) * 128], ident_f[:])
            ah = sb.tile([128, 128], BF16, tag=f"ah{kt_i}", name=f"ah{kt_i}")
            if kt_i % 2:
                nc.vector.tensor_copy(ah[:], tp[:])
            else:
                nc.scalar.copy(ah[:], tp[:])
            ats.append(ah)
        osb = sb.tile([128, D], F32, tag="osb")
        for h in range(2):
            o_ps = ps.tile([128, 512], F32, tag="o_ps")
            hs = slice(h * 512, (h + 1) * 512)
            for kt_i in range(NT):
                nc.tensor.matmul(o_ps[:], ats[kt_i][:], vres[kt_i][:, hs],
                                 start=(kt_i == 0), stop=(kt_i == NT - 1))
            nc.vector.tensor_scalar_mul(osb[:, hs], o_ps[:], iz[:])
        nc.gpsimd.dma_start(out=out_dram.ap()[qi * 128:(qi + 1) * 128, :], in_=osb[:])


def build():
    nc = bacc.Bacc()
    f1 = nc.declare_dram_parameter("feature1", [N, D], F32, isOutput=False)
    f2 = nc.declare_dram_parameter("feature2", [N, D], F32, isOutput=False)
    w = nc.declare_dram_parameter("w_qkv", [D, 3 * D], F32, isOutput=False)
    out1 = nc.declare_dram_parameter("out1", [N, D], F32, isOutput=True)
    out2 = nc.declare_dram_parameter("out2", [N, D], F32, isOutput=True)

    q1t = nc.dram_tensor("q1t", [D, N], F32)
    k1t = nc.dram_tensor("k1t", [D, N], F32)
    v1 = nc.dram_tensor("v1", [N, D], BF16)
    q2t = nc.dram_tensor("q2t", [D, N], F32)
    k2t = nc.dram_tensor("k2t", [D, N], F32)
    v2 = nc.dram_tensor("v2", [N, D], BF16)

    with TileContext(nc) as tc:
        with tc.tile_pool(name="const", bufs=1) as constp:
            ident_f = constp.tile([128, 128], F32, tag="id_f")
            make_identity(nc, ident_f[:])

            with (
                tc.tile_pool(name="wpool", bufs=1) as wp,
                tc.tile_pool(name="apool", bufs=1) as asb,
                tc.tile_pool(name="apsum", bufs=2, space="PSUM") as aps,
            ):
                # first supertile's x loads go out ahead of the weight DMAs
                # so the PE can start transposing immediately
                xs0 = _load_x(nc, asb, f1, 0)
                wqk = []
                for di in range(DT):
                    wt = wp.tile([128, 2048], F32, tag=f"w{di}", name=f"w{di}")
                    nc.sync.dma_start(
                        out=wt[:], in_=w.ap()[di * 128:(di + 1) * 128, 0:2048])
                    wqk.append(wt)
                wv = []
                for di in range(DT):
                    wt = asb.tile([128, 1024], F32, tag="wvtmp", bufs=2)
                    nc.sync.dma_start(
                        out=wt[:], in_=w.ap()[di * 128:(di + 1) * 128, 2048:3072])
                    wh = wp.tile([128, 1024], BF16, tag=f"wvh{di}", name=f"wvh{di}")
                    nc.vector.tensor_copy(wh[:], wt[:])
                    wv.append(wh)
                _phase_a(nc, (asb, aps), f1, ident_f, wqk, wv, q1t, k1t, v1,
                         xs0=xs0)
                _phase_a(nc, (asb, aps), f2, ident_f, wqk, wv, q2t, k2t, v2)

            with (
                tc.tile_pool(name="bpool", bufs=1) as bsb,
                tc.tile_pool(name="bwork", bufs=2) as bwk,
                tc.tile_pool(name="bpsum", bufs=2, space="PSUM") as bps,
            ):
                # dir2 first: its residents (k1t, v1) are ready after
                # phase A(f1), so their loads overlap phase A(f2) compute
                _phase_b(nc, (bsb, bwk, bps), ident_f, q2t, k1t, v1, out2)
                _phase_b(nc, (bsb, bwk, bps), ident_f, q1t, k2t, v2, out1)
    return nc


_NC_CACHE = None


def _get_nc():
    global _NC_CACHE
    if _NC_CACHE is None:
        _NC_CACHE = build()
        _NC_CACHE.finalize()
    return _NC_CACHE


def kernel(feature1, feature2, W_qkv, topk):
    assert int(topk) == TOPK, f"kernel hardcodes topk=16, got {topk}"
    f1 = np.ascontiguousarray(np.asarray(feature1), dtype=np.float32)
    f2 = np.ascontiguousarray(np.asarray(feature2), dtype=np.float32)
    w = np.ascontiguousarray(np.asarray(W_qkv), dtype=np.float32)
    assert f1.shape == (B, N, D) and f2.shape == (B, N, D) and w.shape == (D, 3 * D)

    nc = _get_nc()
    in_maps = [{"feature1": f1[b], "feature2": f2[b], "w_qkv": w} for b in range(B)]
    try:
        res = run_bass_kernel_spmd(nc, in_maps, list(range(B))).results
    except Exception:
        # transient device faults have been observed; one retry on a fresh
        # execution usually clears them
        res = run_bass_kernel_spmd(nc, in_maps, list(range(B))).results
    o1 = np.stack([res[b]["out1"] for b in range(B)]).astype(np.float32)
    o2 = np.stack([res[b]["out2"] for b in range(B)]).astype(np.float32)
    return o1, o2


if __name__ == "__main__":
    f1 = np.load("/root/problem/cache/f1.npy")
    f2 = np.load("/root/problem/cache/f2.npy")
    w = np.load("/root/problem/cache/W.npy")
    o1, o2 = kernel(f1, f2, w, 16)
    r1 = np.load("/root/problem/cache/r1.npy")
    r2 = np.load("/root/problem/cache/r2.npy")
    for nm, o, r in (("2to1", o1, r1), ("1to2", o2, r2)):
        err = np.abs(o - r).max()
        rel = err / np.abs(r).max()
        print(f"{nm}: absmax_err={err:.3e} rel={rel:.3e}")


# revision 12
# speedup vs baseline: 7.4635x; 1.0162x over previous
"""Trainium2 Bass kernel for nn_Bidir_Attention (top-k masked bidirectional
cross-attention).

Data-parallel over batch: each of the 8 NeuronCores processes one batch
element end-to-end (QKV GEMM, scores, softmax, exact top-16 mask via
max8+match_replace, masked AV). W_qkv is replicated.

Precision policy (validated against the reference's jax-on-neuron run):
- The S path (x -> Q,K -> S) is full fp32 PE matmul with d-ascending PSUM
  accumulation, bit-matching the reference's compiled matmuls. This is
  mandatory: the top-16 selection flips on ~1e-6 S perturbations and a
  single flipped row can exceed the 2e-2 gate.
- The V path (V GEMM, A transpose, AV) is single bf16: it only scales the
  output smoothly (measured worst-case 4.6e-3 rel in simulation).

Self-contained: hardcodes B=8, N=2048, D=1024, topk=16.
"""

import sys

import numpy as np

for _p in ("/opt/trn_rl_repo", "/root/.axon_site/_ro/trn_rl_repo"):
    if _p not in sys.path:
        sys.path.append(_p)

import concourse.bacc as bacc
import concourse.mybir as mybir
from concourse.tile import TileContext
from concourse.masks import make_identity
from concourse.bass_utils import run_bass_kernel_spmd

B = 8
N = 2048
D = 1024
NT = N // 128          # 16 row tiles
DT = D // 128          # 8 contraction tiles
TOPK = 16
SCALE = float(1.0 / np.sqrt(D))
NEG = -1e30
F32 = mybir.dt.float32
BF16 = mybir.dt.bfloat16


def _load_x(nc, sb, x_dram, j):
    """Issue the 4 x-tile DMAs for supertile j."""
    xs = []
    for nsub in range(4):
        x = sb.tile([128, D], F32, tag=f"x{nsub}", bufs=2)
        nc.sync.dma_start(
            out=x[:], in_=x_dram.ap()[j * 512 + nsub * 128: j * 512 + (nsub + 1) * 128, :])
        xs.append(x)
    return xs


def _phase_a(nc, pools, x_dram, ident_r, wqk, wv, qt_dram, kt_dram, v_dram,
             xs0=None):
    """QKV GEMM for one feature: writes Q^T (pre-scaled, fp32), K^T (fp32,
    both [D,N]) and V (bf16, [N,D]) to DRAM scratch. x is consumed
    transposed via PE. The Q/K path is bit-exact fp32; V is single bf16."""
    sb, ps = pools
    for j in range(4):                      # supertiles of 512 rows
        xs = xs0 if (j == 0 and xs0 is not None) else _load_x(nc, sb, x_dram, j)
        xT = sb.tile([128, DT, 512], F32, tag="xT", bufs=2)
        xh = sb.tile([128, DT, 512], BF16, tag="xh", bufs=2)
        for nsub in range(4):
            for di in range(DT):
                tp = ps.tile([128, 128], F32, tag="tp")
                nc.tensor.transpose(tp[:], xs[nsub][:, di * 128:(di + 1) * 128], ident_r[:])
                sl = (di, slice(nsub * 128, (nsub + 1) * 128))
                if (nsub * DT + di) % 2:
                    nc.vector.tensor_copy(xT[:, sl[0], sl[1]], tp[:])
                else:
                    nc.scalar.copy(xT[:, sl[0], sl[1]], tp[:])
                nc.vector.tensor_copy(xh[:, sl[0], sl[1]], tp[:])
        # Q^T and K^T: [dout 128-tile, n 512] pieces — fp32, d-ascending
        for t in range(16):
            qk_ps = ps.tile([128, 512], F32, tag="qk_ps")
            for di in range(DT):
                nc.tensor.matmul(qk_ps[:], wqk[di][:, t * 128:(t + 1) * 128],
                                 xT[:, di, :],
                                 start=(di == 0), stop=(di == DT - 1))
            o = sb.tile([128, 512], F32, tag="qko")
            if t < 8:
                nc.scalar.mul(o[:], qk_ps[:], SCALE)   # fold in 1/sqrt(D)
                dst = qt_dram
                r0 = t * 128
            else:
                nc.vector.tensor_copy(o[:], qk_ps[:])
                dst = kt_dram
                r0 = (t - 8) * 128
            nc.gpsimd.dma_start(
                out=dst.ap()[r0:r0 + 128, j * 512:(j + 1) * 512], in_=o[:])
        # V: natural layout [n 128-tile, dout 512] pieces — single bf16
        for nsub in range(4):
            for c in range(2):
                v_ps = ps.tile([128, 512], F32, tag="v_ps")
                nsl = slice(nsub * 128, (nsub + 1) * 128)
                for di in range(DT):
                    nc.tensor.matmul(v_ps[:], xh[:, di, nsl],
                                     wv[di][:, c * 512:(c + 1) * 512],
                                     start=(di == 0), stop=(di == DT - 1))
                o = sb.tile([128, 512], BF16, tag="vo")
                if (nsub * 2 + c) % 2:
                    nc.vector.tensor_copy(o[:], v_ps[:])
                else:
                    nc.scalar.copy(o[:], v_ps[:])
                nc.gpsimd.dma_start(
                    out=v_dram.ap()[j * 512 + nsub * 128: j * 512 + (nsub + 1) * 128,
                                    c * 512:(c + 1) * 512],
                    in_=o[:])


def _phase_b(nc, pools, ident_f, qt_dram, kt_dram, v_dram, out_dram):
    """One attention direction: S = Q^T.T @ K^T (pre-scaled, fp32), softmax
    row stats, exact top-16 mask, masked AV in bf16, 1/Z renormalization."""
    sbr, sb, ps = pools
    # residents — spread across the two HWDGE rings (sync/scalar) plus the
    # gpsimd SWDGE ring so the phase-boundary reload isn't serialized on one
    # DMA queue
    kt = []
    for di in range(DT):
        t = sbr.tile([128, N], F32, tag=f"kt{di}", name=f"kt{di}")
        eng = nc.sync if di % 2 == 0 else nc.scalar
        eng.dma_start(out=t[:], in_=kt_dram.ap()[di * 128:(di + 1) * 128, :])
        kt.append(t)
    vres = []
    for nt in range(NT):
        t = sbr.tile([128, D], BF16, tag=f"vh{nt}", name=f"vh{nt}")
        nc.gpsimd.dma_start(out=t[:], in_=v_dram.ap()[nt * 128:(nt + 1) * 128, :])
        vres.append(t)

    for qi in range(NT):
        # all 8 Q^T blocks for this q-tile in one 512KB DMA:
        # qt_dram[(di p), qcols] -> [p, di, qcols]
        qts = sb.tile([128, DT, 128], F32, tag="qts", bufs=2)
        nc.sync.dma_start(
            out=qts[:],
            in_=qt_dram.ap()[:, qi * 128:(qi + 1) * 128]
                .rearrange("(di p) c -> p di c", p=128))
        ssb = sb.tile([128, N], F32, tag="ssb")
        for half in range(2):
            s_ps = ps.tile([128, N // 2], F32, tag="s_ps", bufs=2)
            for di in range(DT):
                for c in range(2):
                    nc.tensor.matmul(s_ps[:, c * 512:(c + 1) * 512], qts[:, di, :],
                                     kt[di][:, half * 1024 + c * 512:
                                            half * 1024 + (c + 1) * 512],
                                     start=(di == 0), stop=(di == DT - 1))
            nc.vector.tensor_copy(ssb[:, half * 1024:(half + 1) * 1024], s_ps[:])

        m0 = sb.tile([128, 8], F32, tag="m0")
        nc.vector.max(out=m0[:], in_=ssb[:])
        nm = sb.tile([128, 1], F32, tag="nm")
        nc.vector.tensor_scalar_mul(nm[:], m0[:, 0:1], -1.0)
        p = sb.tile([128, N], F32, tag="p")
        z = sb.tile([128, 1], F32, tag="z")
        nc.scalar.activation(p[:], ssb[:], mybir.ActivationFunctionType.Exp,
                             bias=nm[:], scale=1.0, accum_out=z[:])
        iz = sb.tile([128, 1], F32, tag="iz")
        nc.vector.reciprocal(iz[:], z[:])
        # exact top-16: two rounds of max8 + match_replace (in place on ssb,
        # which the Exp above has already consumed)
        nc.vector.match_replace(out=ssb[:], in_to_replace=m0[:], in_values=ssb[:],
                                imm_value=NEG)
        m8 = sb.tile([128, 8], F32, tag="m8")
        nc.vector.max(out=m8[:], in_=ssb[:])
        nc.vector.match_replace(out=ssb[:], in_to_replace=m8[:], in_values=ssb[:],
                                imm_value=NEG)
        # A = exp(S - m) where selected else 0   (in place on p)
        nc.vector.scalar_tensor_tensor(out=p[:], in0=ssb[:], scalar=NEG, in1=p[:],
                                       op0=mybir.AluOpType.is_equal,
                                       op1=mybir.AluOpType.mult)
        # transpose A tiles for the AV matmul (fp32 PE transpose, cast to
        # bf16 during the PSUM->SBUF copy)
        ats = []
        for kt_i in range(NT):
            tp = ps.tile([128, 128], F32, tag="tp2")
            nc.tensor.transpose(tp[:], p[:, kt_i * 128:(kt_i + 1) * 128], ident_f[:])
            ah = sb.tile([128, 128], BF16, tag=f"ah{kt_i}", name=f"ah{kt_i}")
            if kt_i % 2:
                nc.vector.tensor_copy(ah[:], tp[:])
            else:
                nc.scalar.copy(ah[:], tp[:])
            ats.append(ah)
        osb = sb.tile([128, D], F32, tag="osb")
        for h in range(2):
            o_ps = ps.tile([128, 512], F32, tag="o_ps")
            hs = slice(h * 512, (h + 1) * 512)
            for kt_i in range(NT):
                nc.tensor.matmul(o_ps[:], ats[kt_i][:], vres[kt_i][:, hs],
                                 start=(kt_i == 0), stop=(kt_i == NT - 1))
            nc.vector.tensor_scalar_mul(osb[:, hs], o_ps[:], iz[:])
        nc.gpsimd.dma_start(out=out_dram.ap()[qi * 128:(qi + 1) * 128, :], in_=osb[:])


def build():
    nc = bacc.Bacc()
    f1 = nc.declare_dram_parameter("feature1", [N, D], F32, isOutput=False)
    f2 = nc.declare_dram_parameter("feature2", [N, D], F32, isOutput=False)
    w = nc.declare_dram_parameter("w_qkv", [D, 3 * D], F32, isOutput=False)
    out1 = nc.declare_dram_parameter("out1", [N, D], F32, isOutput=True)
    out2 = nc.declare_dram_parameter("out2", [N, D], F32, isOutput=True)

    q1t = nc.dram_tensor("q1t", [D, N], F32)
    k1t = nc.dram_tensor("k1t", [D, N], F32)
    v1 = nc.dram_tensor("v1", [N, D], BF16)
    q2t = nc.dram_tensor("q2t", [D, N], F32)
    k2t = nc.dram_tensor("k2t", [D, N], F32)
    v2 = nc.dram_tensor("v2", [N, D], BF16)

    with TileContext(nc) as tc:
        with tc.tile_pool(name="const", bufs=1) as constp:
            ident_f = constp.tile([128, 128], F32, tag="id_f")
            make_identity(nc, ident_f[:])

            with (
                tc.tile_pool(name="wpool", bufs=1) as wp,
                tc.tile_pool(name="apool", bufs=1) as asb,
                tc.tile_pool(name="apsum", bufs=2, space="PSUM") as aps,
            ):
                # first supertile's x loads go out ahead of the weight DMAs
                # so the PE can start transposing immediately
                xs0 = _load_x(nc, asb, f1, 0)
                wqk = []
                for di in range(DT):
                    wt = wp.tile([128, 2048], F32, tag=f"w{di}", name=f"w{di}")
                    nc.sync.dma_start(
                        out=wt[:], in_=w.ap()[di * 128:(di + 1) * 128, 0:2048])
                    wqk.append(wt)
                wv = []
                for di in range(DT):
                    wt = asb.tile([128, 1024], F32, tag="wvtmp", bufs=2)
                    nc.sync.dma_start(
                        out=wt[:], in_=w.ap()[di * 128:(di + 1) * 128, 2048:3072])
                    wh = wp.tile([128, 1024], BF16, tag=f"wvh{di}", name=f"wvh{di}")
                    nc.vector.tensor_copy(wh[:], wt[:])
                    wv.append(wh)
                _phase_a(nc, (asb, aps), f1, ident_f, wqk, wv, q1t, k1t, v1,
                         xs0=xs0)
                _phase_a(nc, (asb, aps), f2, ident_f, wqk, wv, q2t, k2t, v2)

            with (
                tc.tile_pool(name="bpool", bufs=1) as bsb,
                tc.tile_pool(name="bwork", bufs=2) as bwk,
                tc.tile_pool(name="bpsum", bufs=2, space="PSUM") as bps,
            ):
                # dir2 first: its residents (k1t, v1) are ready after
                # phase A(f1), so their loads overlap phase A(f2) compute
                _phase_b(nc, (bsb, bwk, bps), ident_f, q2t, k1t, v1, out2)
                _phase_b(nc, (bsb, bwk, bps), ident_f, q1t, k2t, v2, out1)
    return nc


_NC_CACHE = None


def _get_nc():
    global _NC_CACHE
    if _NC_CACHE is None:
        _NC_CACHE = build()
        _NC_CACHE.finalize()
    return _NC_CACHE


def kernel(feature1, feature2, W_qkv, topk):
    assert int(topk) == TOPK, f"kernel hardcodes topk=16, got {topk}"
    f1 = np.ascontiguousarray(np.asarray(feature1), dtype=np.float32)
    f2 = np.ascontiguousarray(np.asarray(feature2), dtype=np.float32)
    w = np.ascontiguousarray(np.asarray(W_qkv), dtype=np.float32)
    assert f1.shape == (B, N, D) and f2.shape == (B, N, D) and w.shape == (D, 3 * D)

    nc = _get_nc()
    in_maps = [{"feature1": f1[b], "feature2": f2[b], "w_qkv": w} for b in range(B)]
    try:
        res = run_bass_kernel_spmd(nc, in_maps, list(range(B))).results
    except Exception:
        # transient device faults have been observed; one retry on a fresh
        # execution usually clears them
        res = run_bass_kernel_spmd(nc, in_maps, list(range(B))).results
    o1 = np.stack([res[b]["out1"] for b in range(B)]).astype(np.float32)
    o2 = np.stack([res[b]["out2"] for b in range(B)]).astype(np.float32)
    return o1, o2


if __name__ == "__main__":
    f1 = np.load("/root/problem/cache/f1.npy")
    f2 = np.load("/root/problem/cache/f2.npy")
    w = np.load("/root/problem/cache/W.npy")
    o1, o2 = kernel(f1, f2, w, 16)
    r1 = np.load("/root/problem/cache/r1.npy")
    r2 = np.load("/root/problem/cache/r2.npy")
    for nm, o, r in (("2to1", o1, r1), ("1to2", o2, r2)):
        err = np.abs(o - r).max()
        rel = err / np.abs(r).max()
        print(f"{nm}: absmax_err={err:.3e} rel={rel:.3e}")


# revision 15
# speedup vs baseline: 7.8824x; 1.0561x over previous
"""Trainium2 Bass kernel for nn_Bidir_Attention (top-k masked bidirectional
cross-attention).

Data-parallel over batch: each of the 8 NeuronCores processes one batch
element end-to-end (QKV GEMM, scores, softmax, exact top-16 mask via
max8+match_replace, masked AV). W_qkv is replicated.

Precision policy (validated against the reference's jax-on-neuron run):
- The S path (x -> Q,K -> S) is full fp32 PE matmul with d-ascending PSUM
  accumulation, bit-matching the reference's compiled matmuls. This is
  mandatory: the top-16 selection flips on ~1e-6 S perturbations and a
  single flipped row can exceed the 2e-2 gate.
- The V path (V GEMM, A transpose, AV) is single bf16: it only scales the
  output smoothly (measured worst-case 4.6e-3 rel in simulation).

Self-contained: hardcodes B=8, N=2048, D=1024, topk=16.
"""

import sys

import numpy as np

for _p in ("/opt/trn_rl_repo", "/root/.axon_site/_ro/trn_rl_repo"):
    if _p not in sys.path:
        sys.path.append(_p)

import concourse.bacc as bacc
import concourse.mybir as mybir
from concourse.tile import TileContext
from concourse.masks import make_identity
from concourse.bass_utils import run_bass_kernel_spmd

B = 8
N = 2048
D = 1024
NT = N // 128          # 16 row tiles
DT = D // 128          # 8 contraction tiles
TOPK = 16
SCALE = float(1.0 / np.sqrt(D))
NEG = -1e30
F32 = mybir.dt.float32
BF16 = mybir.dt.bfloat16


def _load_x(nc, sb, x_dram, j):
    """Issue the 4 x-tile DMAs for supertile j."""
    xs = []
    for nsub in range(4):
        x = sb.tile([128, D], F32, tag=f"x{nsub}", bufs=2)
        nc.sync.dma_start(
            out=x[:], in_=x_dram.ap()[j * 512 + nsub * 128: j * 512 + (nsub + 1) * 128, :])
        xs.append(x)
    return xs


def _phase_a(nc, pools, x_dram, ident_r, wqk, wv, qt_dram, kt_dram, v_dram,
             xs0=None):
    """QKV GEMM for one feature: writes Q^T (pre-scaled, fp32), K^T (fp32,
    both [D,N]) and V (bf16, [N,D]) to DRAM scratch. x is consumed
    transposed via PE. The Q/K path is bit-exact fp32; V is single bf16."""
    sb, ps = pools
    for j in range(4):                      # supertiles of 512 rows
        xs = xs0 if (j == 0 and xs0 is not None) else _load_x(nc, sb, x_dram, j)
        xT = sb.tile([128, DT, 512], F32, tag="xT", bufs=2)
        xh = sb.tile([128, DT, 512], BF16, tag="xh", bufs=2)
        for nsub in range(4):
            for di in range(DT):
                tp = ps.tile([128, 128], F32, tag="tp", bufs=4)
                nc.tensor.transpose(tp[:], xs[nsub][:, di * 128:(di + 1) * 128], ident_r[:])
                sl = (di, slice(nsub * 128, (nsub + 1) * 128))
                if (nsub * DT + di) % 2:
                    nc.vector.tensor_copy(xT[:, sl[0], sl[1]], tp[:])
                else:
                    nc.scalar.copy(xT[:, sl[0], sl[1]], tp[:])
                nc.vector.tensor_copy(xh[:, sl[0], sl[1]], tp[:])
        # Q^T and K^T: [dout 128-tile, n 512] pieces — fp32, d-ascending
        for t in range(16):
            qk_ps = ps.tile([128, 512], F32, tag="qk_ps")
            for di in range(DT):
                nc.tensor.matmul(qk_ps[:], wqk[di][:, t * 128:(t + 1) * 128],
                                 xT[:, di, :],
                                 start=(di == 0), stop=(di == DT - 1))
            o = sb.tile([128, 512], F32, tag="qko")
            if t < 8:
                nc.scalar.mul(o[:], qk_ps[:], SCALE)   # fold in 1/sqrt(D)
                dst = qt_dram
                r0 = t * 128
            else:
                nc.vector.tensor_copy(o[:], qk_ps[:])
                dst = kt_dram
                r0 = (t - 8) * 128
            nc.gpsimd.dma_start(
                out=dst.ap()[r0:r0 + 128, j * 512:(j + 1) * 512], in_=o[:])
        # V: natural layout [n 128-tile, dout 512] pieces — single bf16
        for nsub in range(4):
            for c in range(2):
                v_ps = ps.tile([128, 512], F32, tag="v_ps")
                nsl = slice(nsub * 128, (nsub + 1) * 128)
                for di in range(DT):
                    nc.tensor.matmul(v_ps[:], xh[:, di, nsl],
                                     wv[di][:, c * 512:(c + 1) * 512],
                                     start=(di == 0), stop=(di == DT - 1))
                o = sb.tile([128, 512], BF16, tag="vo")
                if (nsub * 2 + c) % 2:
                    nc.vector.tensor_copy(o[:], v_ps[:])
                else:
                    nc.scalar.copy(o[:], v_ps[:])
                nc.gpsimd.dma_start(
                    out=v_dram.ap()[j * 512 + nsub * 128: j * 512 + (nsub + 1) * 128,
                                    c * 512:(c + 1) * 512],
                    in_=o[:])


def _phase_b(nc, pools, ident_f, qt_dram, kt_dram, v_dram, out_dram):
    """One attention direction: S = Q^T.T @ K^T (pre-scaled, fp32), softmax
    row stats, exact top-16 mask, masked AV in bf16, 1/Z renormalization."""
    sbr, sb, ps = pools

    def _load_qts(qi):
        # all 8 Q^T blocks for this q-tile in one 512KB DMA:
        # qt_dram[(di p), qcols] -> [p, di, qcols]
        qts = sb.tile([128, DT, 128], F32, tag="qts", bufs=2)
        nc.sync.dma_start(
            out=qts[:],
            in_=qt_dram.ap()[:, qi * 128:(qi + 1) * 128]
                .rearrange("(di p) c -> p di c", p=128))
        return qts

    # first q-tile's Q blocks go out ahead of the resident reload so the
    # first S matmul only waits for kt[0], not the whole 10MB
    qts0 = _load_qts(0)
    # residents — spread across the two HWDGE rings (sync/scalar) plus the
    # gpsimd SWDGE ring so the phase-boundary reload isn't serialized on one
    # DMA queue; issued in consumption order (kt[0] first on each ring)
    kt = []
    for di in range(DT):
        t = sbr.tile([128, N], F32, tag=f"kt{di}", name=f"kt{di}")
        eng = nc.scalar if di % 2 == 0 else nc.sync
        eng.dma_start(out=t[:], in_=kt_dram.ap()[di * 128:(di + 1) * 128, :])
        kt.append(t)
    vres = []
    for nt in range(NT):
        t = sbr.tile([128, D], BF16, tag=f"vh{nt}", name=f"vh{nt}")
        nc.gpsimd.dma_start(out=t[:], in_=v_dram.ap()[nt * 128:(nt + 1) * 128, :])
        vres.append(t)

    for qi in range(NT):
        qts = qts0 if qi == 0 else _load_qts(qi)
        ssb = sb.tile([128, N], F32, tag="ssb")
        for half in range(2):
            s_ps = ps.tile([128, N // 2], F32, tag="s_ps", bufs=2)
            for di in range(DT):
                for c in range(2):
                    nc.tensor.matmul(s_ps[:, c * 512:(c + 1) * 512], qts[:, di, :],
                                     kt[di][:, half * 1024 + c * 512:
                                            half * 1024 + (c + 1) * 512],
                                     start=(di == 0), stop=(di == DT - 1))
            nc.vector.tensor_copy(ssb[:, half * 1024:(half + 1) * 1024], s_ps[:])

        m0 = sb.tile([128, 8], F32, tag="m0")
        nc.vector.max(out=m0[:], in_=ssb[:])
        nm = sb.tile([128, 1], F32, tag="nm")
        nc.vector.tensor_scalar_mul(nm[:], m0[:, 0:1], -1.0)
        p = sb.tile([128, N], F32, tag="p")
        z = sb.tile([128, 1], F32, tag="z")
        nc.scalar.activation(p[:], ssb[:], mybir.ActivationFunctionType.Exp,
                             bias=nm[:], scale=1.0, accum_out=z[:])
        iz = sb.tile([128, 1], F32, tag="iz")
        nc.vector.reciprocal(iz[:], z[:])
        # exact top-16: knock out the top-8 (into a scratch copy), max8 of the
        # rest gives ranks 9-16; the smallest of those is the row threshold
        s2 = sb.tile([128, N], F32, tag="s2")
        nc.vector.match_replace(out=s2[:], in_to_replace=m0[:], in_values=ssb[:],
                                imm_value=NEG)
        m8 = sb.tile([128, 8], F32, tag="m8")
        nc.vector.max(out=m8[:], in_=s2[:])
        # A = exp(S - m) where S >= threshold else 0   (in place on p)
        nc.vector.scalar_tensor_tensor(out=p[:], in0=ssb[:], scalar=m8[:, 7:8],
                                       in1=p[:],
                                       op0=mybir.AluOpType.is_ge,
                                       op1=mybir.AluOpType.mult)
        # transpose A tiles for the AV matmul (fp32 PE transpose, cast to
        # bf16 during the PSUM->SBUF copy)
        ats = []
        for kt_i in range(NT):
            tp = ps.tile([128, 128], F32, tag="tp2")
            nc.tensor.transpose(tp[:], p[:, kt_i * 128:(kt_i + 1) * 128], ident_f[:])
            ah = sb.tile([128, 128], BF16, tag=f"ah{kt_i}", name=f"ah{kt_i}")
            if kt_i % 2:
                nc.vector.tensor_copy(ah[:], tp[:])
            else:
                nc.scalar.copy(ah[:], tp[:])
            ats.append(ah)
        osb = sb.tile([128, D], F32, tag="osb")
        for h in range(2):
            o_ps = ps.tile([128, 512], F32, tag="o_ps")
            hs = slice(h * 512, (h + 1) * 512)
            for kt_i in range(NT):
                nc.tensor.matmul(o_ps[:], ats[kt_i][:], vres[kt_i][:, hs],
                                 start=(kt_i == 0), stop=(kt_i == NT - 1))
            nc.vector.tensor_scalar_mul(osb[:, hs], o_ps[:], iz[:])
        nc.gpsimd.dma_start(out=out_dram.ap()[qi * 128:(qi + 1) * 128, :], in_=osb[:])


def build():
    nc = bacc.Bacc()
    f1 = nc.declare_dram_parameter("feature1", [N, D], F32, isOutput=False)
    f2 = nc.declare_dram_parameter("feature2", [N, D], F32, isOutput=False)
    w = nc.declare_dram_parameter("w_qkv", [D, 3 * D], F32, isOutput=False)
    out1 = nc.declare_dram_parameter("out1", [N, D], F32, isOutput=True)
    out2 = nc.declare_dram_parameter("out2", [N, D], F32, isOutput=True)

    q1t = nc.dram_tensor("q1t", [D, N], F32)
    k1t = nc.dram_tensor("k1t", [D, N], F32)
    v1 = nc.dram_tensor("v1", [N, D], BF16)
    q2t = nc.dram_tensor("q2t", [D, N], F32)
    k2t = nc.dram_tensor("k2t", [D, N], F32)
    v2 = nc.dram_tensor("v2", [N, D], BF16)

    with TileContext(nc) as tc:
        with tc.tile_pool(name="const", bufs=1) as constp:
            ident_f = constp.tile([128, 128], F32, tag="id_f")
            make_identity(nc, ident_f[:])

            with (
                tc.tile_pool(name="wpool", bufs=1) as wp,
                tc.tile_pool(name="apool", bufs=1) as asb,
                tc.tile_pool(name="apsum", bufs=2, space="PSUM") as aps,
            ):
                # first supertile's x loads go out ahead of the weight DMAs
                # so the PE can start transposing immediately
                xs0 = _load_x(nc, asb, f1, 0)
                wqk = []
                for di in range(DT):
                    wt = wp.tile([128, 2048], F32, tag=f"w{di}", name=f"w{di}")
                    nc.sync.dma_start(
                        out=wt[:], in_=w.ap()[di * 128:(di + 1) * 128, 0:2048])
                    wqk.append(wt)
                wv = []
                for di in range(DT):
                    wt = asb.tile([128, 1024], F32, tag="wvtmp", bufs=2)
                    nc.sync.dma_start(
                        out=wt[:], in_=w.ap()[di * 128:(di + 1) * 128, 2048:3072])
                    wh = wp.tile([128, 1024], BF16, tag=f"wvh{di}", name=f"wvh{di}")
                    nc.vector.tensor_copy(wh[:], wt[:])
                    wv.append(wh)
                _phase_a(nc, (asb, aps), f1, ident_f, wqk, wv, q1t, k1t, v1,
                         xs0=xs0)
                _phase_a(nc, (asb, aps), f2, ident_f, wqk, wv, q2t, k2t, v2)

            with (
                tc.tile_pool(name="bpool", bufs=1) as bsb,
                tc.tile_pool(name="bwork", bufs=2) as bwk,
                tc.tile_pool(name="bpsum", bufs=2, space="PSUM") as bps,
            ):
                # dir2 first: its residents (k1t, v1) are ready after
                # phase A(f1), so their loads overlap phase A(f2) compute
                _phase_b(nc, (bsb, bwk, bps), ident_f, q2t, k1t, v1, out2)
                _phase_b(nc, (bsb, bwk, bps), ident_f, q1t, k2t, v2, out1)
    return nc


_NC_CACHE = None


def _get_nc():
    global _NC_CACHE
    if _NC_CACHE is None:
        _NC_CACHE = build()
        _NC_CACHE.finalize()
    return _NC_CACHE


def kernel(feature1, feature2, W_qkv, topk):
    assert int(topk) == TOPK, f"kernel hardcodes topk=16, got {topk}"
    f1 = np.ascontiguousarray(np.asarray(feature1), dtype=np.float32)
    f2 = np.ascontiguousarray(np.asarray(feature2), dtype=np.float32)
    w = np.ascontiguousarray(np.asarray(W_qkv), dtype=np.float32)
    assert f1.shape == (B, N, D) and f2.shape == (B, N, D) and w.shape == (D, 3 * D)

    nc = _get_nc()
    in_maps = [{"feature1": f1[b], "feature2": f2[b], "w_qkv": w} for b in range(B)]
    try:
        res = run_bass_kernel_spmd(nc, in_maps, list(range(B))).results
    except Exception:
        # transient device faults have been observed; one retry on a fresh
        # execution usually clears them
        res = run_bass_kernel_spmd(nc, in_maps, list(range(B))).results
    o1 = np.stack([res[b]["out1"] for b in range(B)]).astype(np.float32)
    o2 = np.stack([res[b]["out2"] for b in range(B)]).astype(np.float32)
    return o1, o2


if __name__ == "__main__":
    f1 = np.load("/root/problem/cache/f1.npy")
    f2 = np.load("/root/problem/cache/f2.npy")
    w = np.load("/root/problem/cache/W.npy")
    o1, o2 = kernel(f1, f2, w, 16)
    r1 = np.load("/root/problem/cache/r1.npy")
    r2 = np.load("/root/problem/cache/r2.npy")
    for nm, o, r in (("2to1", o1, r1), ("1to2", o2, r2)):
        err = np.abs(o - r).max()
        rel = err / np.abs(r).max()
        print(f"{nm}: absmax_err={err:.3e} rel={rel:.3e}")
